# revision 19
# baseline (speedup 1.0000x reference)
"""BERT-base encoder (12 layers) forward for Trainium2, data-parallel over batch.

v3: fp8 DoubleRow GEMMs + XBAR DMA transposes (no PE transposes) + fp8
attention internals + interleaved token-half scheduling.

  - GEMMs in fp8e4 DoubleRow (2 contraction chunks / instr, 0.5 cycles/row):
      QKV:  x_h (w_h + w_l)                    [2-term]
      O/FFN1/FFN2: x_h w_h + x_h w_l + x_l w_h [3-term]
    weights host-prescaled by 32 (lo parts stay normal); 1/32 folded into
    PSUM evictions.
  - all activation transposes (h^T, ctx^T, h1^T) via DMA XBAR transpose
    (bf16), fp8 hi/lo splits from SBUF on Act/DVE/Pool.
  - attention internals fp8 (scores from fp8 q/k, softmax weights and V fp8);
    softmax row-sum fused as a ones-column in V, normalization in eviction.
  - token-half interleave: QKV GEMMs for tokens 0-511 -> attention groups
    0-3 -> QKV for 512-1023 -> groups 4-7, so Act/DVE eviction latency hides
    under PE work.
"""

import numpy as np
import ml_dtypes

import concourse.bass as bass
import concourse.mybir as mybir
import concourse.tile as tile
from concourse import bacc
from concourse.bass_utils import run_bass_kernel_spmd

V, D, L, H, S, B = 30522, 768, 12, 12, 64, 128
DK = D // H            # 64
FF = 4 * D             # 3072
EPS = 1e-5
NCORES = 8
BL = B // NCORES       # 16 sequences per core
T = BL * S             # 1024 tokens per core
P = 128
NT = T // P            # 8 token tiles (= 2-sequence groups)
KD = D // P            # 6 feature tiles
KF = FF // P           # 24 ff tiles
NEG = -1.0e10
WS = 32.0              # host weight pre-scale
ISC = 1.0 / WS
DK1 = DK + 1           # ctx columns + fused rowsum column

F32 = mybir.dt.float32
BF16 = mybir.dt.bfloat16
F8 = mybir.dt.float8e4
I32 = mybir.dt.int32

AF = mybir.ActivationFunctionType
ALU = mybir.AluOpType
DR = mybir.MatmulPerfMode.DoubleRow


def _positional_table():
    pos = np.arange(S, dtype=np.float32)[:, None]
    i = np.arange(0, D, 2, dtype=np.float32)
    arg = pos / (10000.0 ** (2.0 * i / D))
    pe = np.zeros((S, D), dtype=np.float32)
    pe[:, 0::2] = np.sin(arg)
    pe[:, 1::2] = np.cos(arg)
    return pe  # [S, D] f32


def _block_diag_mask():
    m = np.full((P, P), NEG, dtype=np.float32)
    m[:S, :S] = 0.0
    m[S:, S:] = 0.0
    return m


def _build_program(n_layers=L):
    nc = bacc.Bacc("TRN2", target_bir_lowering=False, debug=False,
                   num_devices=NCORES)

    x_idx = nc.dram_tensor("x_idx", [T], I32, kind="ExternalInput").ap()
    h0 = nc.dram_tensor("h0", [T, D], F32, kind="ExternalInput").ap()
    bdm = nc.dram_tensor("bdm", [P, P], F32, kind="ExternalInput").ap()
    wq8 = nc.dram_tensor("wq8", [n_layers, P, 2, KD, D], F8,
                         kind="ExternalInput").ap()
    wk8 = nc.dram_tensor("wk8", [n_layers, P, 2, KD, D], F8,
                         kind="ExternalInput").ap()
    wv8 = nc.dram_tensor("wv8", [n_layers, P, 2, KD, D], F8,
                         kind="ExternalInput").ap()
    wo8 = nc.dram_tensor("wo8", [n_layers, P, 2, KD, D], F8,
                         kind="ExternalInput").ap()
    w18 = nc.dram_tensor("w18", [n_layers, 8, P, 2, 3, KD, P], F8,
                         kind="ExternalInput").ap()
    w28 = nc.dram_tensor("w28", [n_layers, 8, P, 2, 6, 384], F8,
                         kind="ExternalInput").ap()
    out = nc.dram_tensor("out", [T, D], F32, kind="ExternalOutput").ap()

    with tile.TileContext(nc) as tc:
        import contextlib
        ctx = contextlib.ExitStack()
        with ctx:
            const = ctx.enter_context(tc.tile_pool(name="const", bufs=1))
            resid = ctx.enter_context(tc.tile_pool(name="resid", bufs=12))
            f8act = ctx.enter_context(tc.tile_pool(name="f8act", bufs=5))
            qk8 = ctx.enter_context(tc.tile_pool(name="qk8", bufs=2))
            vc = ctx.enter_context(tc.tile_pool(name="vc", bufs=1))
            g8 = ctx.enter_context(tc.tile_pool(name="g8", bufs=2))
            pp = ctx.enter_context(tc.tile_pool(name="pp", bufs=5))
            gB = ctx.enter_context(tc.tile_pool(name="gB", bufs=2))
            hc = ctx.enter_context(tc.tile_pool(name="hc", bufs=2))
            bfT = ctx.enter_context(tc.tile_pool(name="bfT", bufs=1))
            wqk = ctx.enter_context(tc.tile_pool(name="wqk", bufs=2))
            wvo = ctx.enter_context(tc.tile_pool(name="wvo", bufs=2))
            w1p = ctx.enter_context(tc.tile_pool(name="w1p", bufs=2))
            w2p = ctx.enter_context(tc.tile_pool(name="w2p", bufs=3))
            small = ctx.enter_context(tc.tile_pool(name="small", bufs=6))
            psum = ctx.enter_context(
                tc.tile_pool(name="psum", bufs=4, space="PSUM"))
            psum4 = ctx.enter_context(
                tc.tile_pool(name="psum4", bufs=4, space="PSUM"))

            # ---- constants ----
            eps_sb = const.tile([P, 1], F32, tag="eps")
            nc.vector.memset(eps_sb[:], EPS)
            bd_sb = const.tile([P, P], F32, tag="bd")
            nc.sync.dma_start(out=bd_sb[:], in_=bdm[:])

            # ---- embedding (host-precomputed) ----
            h_tiles = []
            for ti in range(NT):
                h = resid.tile([P, D], F32, tag="resid")
                nc.sync.dma_start(out=h[:], in_=h0[ti * P:(ti + 1) * P, :])
                h_tiles.append(h)

            # ---- mask tiles ----
            xg = small.tile([P, NT], I32, tag="xg")
            nc.sync.dma_start(out=xg[:], in_=x_idx.rearrange("(g p) -> p g", p=P))
            am = small.tile([P, NT], F32, tag="am")
            nc.vector.tensor_scalar(out=am[:], in0=xg[:], scalar1=0, scalar2=None,
                                    op0=ALU.is_gt)
            nc.vector.tensor_scalar(out=am[:], in0=am[:], scalar1=1.0,
                                    scalar2=-NEG, op0=ALU.subtract, op1=ALU.mult)
            maskt = const.tile([P, NT, P], BF16, tag="maskt")
            for g in range(NT):
                nc.vector.tensor_scalar(out=maskt[:, g, :], in0=bd_sb[:],
                                        scalar1=am[:, g:g + 1], scalar2=None,
                                        op0=ALU.add)

            def ln_inplace(r):
                st = small.tile([P, 3, 6], F32, tag="st")
                for sg in range(3):
                    nc.vector.bn_stats(out=st[:, sg, :],
                                       in_=r[:, sg * 256:(sg + 1) * 256])
                mv = small.tile([P, 2], F32, tag="mv")
                nc.vector.bn_aggr(out=mv[:], in_=st[:])
                rstd = small.tile([P, 1], F32, tag="rstd")
                nc.scalar.activation(out=rstd[:], in_=mv[:, 1:2],
                                     func=AF.Sqrt, bias=eps_sb[:])
                nc.vector.reciprocal(out=rstd[:], in_=rstd[:])
                nc.vector.tensor_scalar(out=r[:], in0=r[:],
                                        scalar1=mv[:, 0:1], scalar2=rstd[:],
                                        op0=ALU.subtract, op1=ALU.mult)

            # ---- transformer layers ----
            # Software-pipelined emission: attention is split into
            # scores-halves and PV-halves with QKV matmul groups as filler,
            # FFN2(th0) interleaves with FFN1(th1), LN2 is deferred to keep
            # the Act table sequence at Exp|Sqrt|Gelu|Sqrt (4 loads/layer),
            # and next-layer weights + h^T chains are emitted inside the FFN
            # phase so every phase boundary has PE work queued.

            def load_wqkvo(l):
                tiles = {}
                for nm, ap in (("q", wq8), ("k", wk8)):
                    t = wqk.tile([P, 2, KD, D], F8, tag="wqk")
                    nc.sync.dma_start(out=t[:], in_=ap[l])
                    tiles[nm] = t
                for nm, ap in (("v", wv8), ("o", wo8)):
                    t = wvo.tile([P, 2, KD, D], F8, tag="wvo")
                    nc.sync.dma_start(out=t[:], in_=ap[l])
                    tiles[nm] = t
                return tiles

            def emit_hT_chain(h_tiles, xhT, hbT, tis):
                for ti in tis:
                    hb = hc.tile([P, D], BF16, tag="hc")
                    nc.gpsimd.tensor_copy(out=hb[:], in_=h_tiles[ti][:])
                    sl = slice(ti * P, (ti + 1) * P)
                    nc.sync.dma_start(out=hbT[:, :, sl], in_=hb[:],
                                      transpose=True)
                    nc.gpsimd.tensor_copy(out=xhT[:, :, sl],
                                          in_=hbT[:, :, sl])

            cur_w = load_wqkvo(0)
            xhT = f8act.tile([P, KD, T], F8, tag="f8", name="xhT0")
            hbT = bfT.tile([P, KD, T], BF16, tag="bfT", name="hbT0")
            emit_hT_chain(h_tiles, xhT, hbT, range(NT))
            qk_next = (qk8.tile([P, KD, T], F8, tag="qk8", name="qT0"),
                       qk8.tile([P, KD, T], F8, tag="qk8", name="kT0"))
            qk0_pending = None

            for l in range(n_layers):
                qTa, kTa = qk_next
                Vta = vc.tile([P, NT, H * DK1], F8, tag="vta", name=f"Vt{l}")
                ones_v = Vta[:].rearrange("p g (h c) -> p g h c", c=DK1)
                nc.vector.memset(ones_v[:, :, :, DK:DK1], 1.0)
                cta = vc.tile([P, NT, D], BF16, tag="cta", name=f"ctx{l}")
                ch = f8act.tile([P, KD, T], F8, tag="f8", name=f"ch{l}")
                cl = f8act.tile([P, KD, T], F8, tag="f8", name=f"cl{l}")
                cbT = bfT.tile([P, KD, T], BF16, tag="bfT", name=f"cbT{l}")

                def qk_group(w_sb, dstT, j, tc2, xsrc, on_dve=False):
                    def go():
                        tok = slice(tc2 * 512, (tc2 + 1) * 512)
                        ps = psum.tile([P, 512], F32, tag="ps")
                        for v in range(2):
                            for kp in range(3):
                                nc.tensor.matmul(
                                    out=ps[:],
                                    lhsT=w_sb[:, v, 2 * kp:2 * kp + 2,
                                              j * P:(j + 1) * P],
                                    rhs=xsrc[:, 2 * kp:2 * kp + 2, tok],
                                    start=(v == 0 and kp == 0),
                                    stop=(v == 1 and kp == 2),
                                    perf_mode=DR)
                        if on_dve:
                            nc.vector.tensor_scalar(
                                out=dstT[:, j, tok], in0=ps[:],
                                scalar1=ISC, scalar2=None, op0=ALU.mult)
                        else:
                            nc.scalar.activation(out=dstT[:, j, tok],
                                                 in_=ps[:],
                                                 func=AF.Copy, scale=ISC)
                    return go

                def v_group(ti, ncc):
                    def go():
                        ps = psum.tile([P, 384], F32, tag="ps")
                        for v in range(2):
                            for kp in range(3):
                                nc.tensor.matmul(
                                    out=ps[:],
                                    lhsT=xhT[:, 2 * kp:2 * kp + 2,
                                             ti * P:(ti + 1) * P],
                                    rhs=cur_w["v"][:, v, 2 * kp:2 * kp + 2,
                                                   ncc * 384:(ncc + 1) * 384],
                                    start=(v == 0 and kp == 0),
                                    stop=(v == 1 and kp == 2),
                                    perf_mode=DR)
                        nc.vector.tensor_scalar(
                            out=ones_v[:, ti, 6 * ncc:6 * ncc + 6, 0:DK],
                            in0=ps[:].rearrange("p (h c) -> p h c", c=DK),
                            scalar1=ISC, scalar2=None, op0=ALU.mult)
                    return go

                BLKS = ((0, 0, 4), (0, 4, 2), (1, 0, 4), (1, 4, 2))

                def make_attn(g, bi):
                    two, pr0, npr = BLKS[bi]
                    po = two * DK
                    w_ = npr * P
                    st = {}

                    def sc():
                        sps = psum.tile([P, w_], F32, tag="ps",
                                        name=f"sps{l}_{g}_{bi}")
                        for i in range(npr):
                            jt = pr0 + i
                            nc.tensor.matmul(
                                out=sps[:, i * P:(i + 1) * P],
                                lhsT=kTa[po:po + DK, jt, g * P:(g + 1) * P],
                                rhs=qTa[po:po + DK, jt, g * P:(g + 1) * P],
                                start=True, stop=True)
                        sps4 = sps[:].rearrange("p (i c) -> p i c", i=npr)
                        nc.vector.tensor_tensor(
                            out=sps4, in0=sps4,
                            in1=maskt[:, g, None, :].to_broadcast([P, npr, P]),
                            op=ALU.add)
                        pT = pp.tile([P, 512], F8, tag="pt4")
                        nc.scalar.activation(out=pT[:, :w_], in_=sps[:],
                                             func=AF.Exp, scale=0.125)
                        st["pT"] = pT

                    def pv():
                        pT = st["pT"]
                        cps = psum4.tile([P, npr * DK1], F32, tag="ps4",
                                         name=f"cps{l}_{g}_{bi}")
                        for i in range(npr):
                            hh = 2 * (pr0 + i) + two
                            nc.tensor.matmul(
                                out=cps[:, i * DK1:(i + 1) * DK1],
                                lhsT=pT[:, i * P:(i + 1) * P],
                                rhs=Vta[:, g, hh * DK1:(hh + 1) * DK1],
                                start=True, stop=True)
                        cpsv = cps[:].rearrange("p (i c) -> p i c", c=DK1)
                        rsi = small.tile([P, 4], F32, tag="rsi")
                        nc.vector.reciprocal(out=rsi[:, :npr],
                                             in_=cpsv[:, :, DK])
                        cta_v = cta[:, g, :].rearrange(
                            "p (pr two c) -> p two pr c", two=2, c=DK)
                        nc.vector.tensor_tensor(
                            out=cta_v[:, two, pr0:pr0 + npr, :],
                            in0=cpsv[:, :, 0:DK],
                            in1=rsi[:, :npr, None].to_broadcast([P, npr, DK]),
                            op=ALU.mult)
                        if bi == 3:
                            sl = slice(g * P, (g + 1) * P)
                            nc.sync.dma_start(out=cbT[:, :, sl],
                                              in_=cta[:, g, :], transpose=True)
                            nc.scalar.copy(out=ch[:, :, sl],
                                           in_=cbT[:, :, sl])
                            nc.vector.tensor_tensor(out=cl[:, :, sl],
                                                    in0=cbT[:, :, sl],
                                                    in1=ch[:, :, sl],
                                                    op=ALU.subtract)
                    return sc, pv

                def attn_pipe(gs, fill):
                    blocks = [make_attn(g, bi) for g in gs for bi in range(4)]
                    n = len(blocks)
                    fi = iter(fill)
                    blocks[0][0]()
                    blocks[1][0]()
                    for i in range(n):
                        if i % 2 == 0:
                            f = next(fi, None)
                            if f is not None:
                                f()
                        if i + 2 < n:
                            blocks[i + 2][0]()
                        blocks[i][1]()
                    for f in fi:
                        f()

                # [A/B] QKV for token half 0 (layer 0 emits here; later
                # layers emit these as FFN2-th1 fill of the previous layer)
                if qk0_pending is None:
                    qk0_pending = [qk_group(cur_w["q"], qTa, j, 0, xhT,
                                            False) for j in range(KD)]
                    qk0_pending += [qk_group(cur_w["k"], kTa, j, 0, xhT,
                                             False) for j in range(KD)]
                for go in qk0_pending:
                    go()
                for ti in range(4):
                    for ncc in range(2):
                        v_group(ti, ncc)()
                # [C] attention 0-3 with QK half-1 as filler
                fill = [qk_group(cur_w["q"], qTa, j, 1, xhT, False)
                        for j in range(KD)]
                fill += [qk_group(cur_w["k"], kTa, j, 1, xhT, False)
                         for j in range(KD)]
                attn_pipe(range(4), fill)
                # [D] attention 4-7 with V half-1 as filler
                fill = [v_group(ti, ncc) for ti in range(4, 8)
                        for ncc in range(2)]
                attn_pipe(range(4, 8), fill)

                # [E] O-projection + LN1 + h1^T chain per tile
                xh1 = f8act.tile([P, KD, T], F8, tag="f8", name=f"xh1{l}")
                xl1 = f8act.tile([P, KD, T], F8, tag="f8", name=f"xl1{l}")
                h1bT = bfT.tile([P, KD, T], BF16, tag="bfT", name=f"h1bT{l}")

                w1_seq = [fc for fc in range(8)] * 2
                w1_slots = [None] * len(w1_seq)

                def ensure_w1(pos):
                    if pos < len(w1_seq) and w1_slots[pos] is None:
                        t = w1p.tile([P, 2, 3, KD, P], F8, tag="w1")
                        nc.sync.dma_start(out=t[:], in_=w18[l, w1_seq[pos]])
                        w1_slots[pos] = t

                w2_seq = [kc * 2 + ncc for th in range(2)
                          for ncc in range(2) for kc in range(4)]
                w2_slots = [None] * len(w2_seq)

                def ensure_w2(pos):
                    if pos < len(w2_seq) and w2_slots[pos] is None:
                        t = w2p.tile([P, 2, 6, 384], F8, tag="w2")
                        nc.sync.dma_start(out=t[:], in_=w28[l, w2_seq[pos]])
                        w2_slots[pos] = t

                ensure_w1(0)
                ensure_w1(1)
                h1_tiles = []
                for ti in range(NT):
                    r = resid.tile([P, D], F32, tag="resid")
                    for ncc in range(2):
                        ps = psum.tile([P, 384], F32, tag="ps")
                        for a, (xt, v) in enumerate(
                                ((ch, 0), (cl, 0), (ch, 1))):
                            for kp in range(3):
                                nc.tensor.matmul(
                                    out=ps[:],
                                    lhsT=xt[:, 2 * kp:2 * kp + 2,
                                            ti * P:(ti + 1) * P],
                                    rhs=cur_w["o"][:, v, 2 * kp:2 * kp + 2,
                                                   ncc * 384:(ncc + 1) * 384],
                                    start=(a == 0 and kp == 0),
                                    stop=(a == 2 and kp == 2),
                                    perf_mode=DR)
                        nc.vector.scalar_tensor_tensor(
                            out=r[:, ncc * 384:(ncc + 1) * 384],
                            in0=ps[:], scalar=ISC,
                            in1=h_tiles[ti][:, ncc * 384:(ncc + 1) * 384],
                            op0=ALU.mult, op1=ALU.add)
                    ln_inplace(r[:])
                    h1_tiles.append(r)
                    hb = hc.tile([P, D], BF16, tag="hc")
                    nc.gpsimd.tensor_copy(out=hb[:], in_=r[:])
                    sl = slice(ti * P, (ti + 1) * P)
                    nc.sync.dma_start(out=h1bT[:, :, sl], in_=hb[:],
                                      transpose=True)
                    nc.scalar.copy(out=xh1[:, :, sl], in_=h1bT[:, :, sl])
                    nc.vector.tensor_tensor(out=xl1[:, :, sl],
                                            in0=h1bT[:, :, sl],
                                            in1=xh1[:, :, sl],
                                            op=ALU.subtract)

                # FFN pieces
                def ffn1_group(th, fc):
                    def go():
                        tok = slice(th * 512, (th + 1) * 512)
                        w1pos = th * 8 + fc
                        ensure_w1(w1pos + 1)
                        w1c = w1_slots[w1pos]
                        gh8, gl8 = g_tiles[th]
                        for ff in range(3):
                            ft = fc * 3 + ff
                            ps = psum.tile([P, 512], F32, tag="ps")
                            for a, (xt, v) in enumerate(
                                    ((xh1, 0), (xh1, 1), (xl1, 0))):
                                for kp in range(3):
                                    nc.tensor.matmul(
                                        out=ps[:],
                                        lhsT=w1c[:, v, ff,
                                                 2 * kp:2 * kp + 2, :],
                                        rhs=xt[:, 2 * kp:2 * kp + 2, tok],
                                        start=(a == 0 and kp == 0),
                                        stop=(a == 2 and kp == 2),
                                        perf_mode=DR)
                            gBt = gB.tile([P, 512], BF16, tag="gB")
                            nc.scalar.activation(out=gBt[:], in_=ps[:],
                                                 func=AF.Gelu, scale=ISC)
                            nc.scalar.activation(out=gh8[:, ft, :], in_=ps[:],
                                                 func=AF.Gelu, scale=ISC)
                            nc.gpsimd.tensor_sub(out=gl8[:, ft, :],
                                                 in0=gBt[:],
                                                 in1=gh8[:, ft, :])
                    return go

                def ffn2_chunk(th, ncc, kci, pss):
                    def go():
                        w2pos = (th * 2 + ncc) * 4 + kci
                        ensure_w2(w2pos + 1)
                        ensure_w2(w2pos + 2)
                        w2c = w2_slots[w2pos]
                        i8 = w2_seq[w2pos]
                        gh8, gl8 = g_tiles[th]
                        for kkp in range(3):
                            first = (kci == 0 and kkp == 0)
                            last = (kci == 3 and kkp == 2)
                            kt = (i8 // 2) * 6 + 2 * kkp
                            for tt in range(4):
                                for a, (gt, v) in enumerate(
                                        ((gh8, 0), (gl8, 0), (gh8, 1))):
                                    nc.tensor.matmul(
                                        out=pss[tt][:],
                                        lhsT=gt[:, kt:kt + 2,
                                                tt * P:(tt + 1) * P],
                                        rhs=w2c[:, v, 2 * kkp:2 * kkp + 2, :],
                                        start=(first and a == 0),
                                        stop=(last and a == 2),
                                        perf_mode=DR)
                    return go

                def ffn2_evict(th, ncc, pss):
                    def go():
                        for tt in range(4):
                            ti = th * 4 + tt
                            nc.vector.scalar_tensor_tensor(
                                out=rr_tiles[th][tt][:,
                                                     ncc * 384:(ncc + 1) * 384],
                                in0=pss[tt][:], scalar=ISC,
                                in1=h1_tiles[ti][:,
                                                 ncc * 384:(ncc + 1) * 384],
                                op0=ALU.mult, op1=ALU.add)
                    return go

                g_tiles = {}
                rr_tiles = {}
                for th in range(2):
                    g_tiles[th] = (
                        g8.tile([P, KF, 512], F8, tag="g8", name=f"gh{l}_{th}"),
                        g8.tile([P, KF, 512], F8, tag="g8", name=f"gl{l}_{th}"))
                    rr_tiles[th] = [resid.tile([P, D], F32, tag="resid",
                                               name=f"rr{l}_{th}_{tt}")
                                    for tt in range(4)]
                ensure_w2(0)
                ensure_w2(1)

                # [F] FFN1 th0
                for fc in range(8):
                    ffn1_group(0, fc)()

                # [G] FFN2 th0 interleaved with FFN1 th1; prefetch next-layer
                # weights at the start
                if l + 1 < n_layers:
                    next_w = load_wqkvo(l + 1)
                items_a = []
                pss_hold = {}
                for ncc in range(2):
                    pss = [psum4.tile([P, 384], F32, tag="ps4",
                                      name=f"pss{l}_0_{ncc}_{j}")
                           for j in range(4)]
                    pss_hold[ncc] = pss
                    for kci in range(4):
                        items_a.append(ffn2_chunk(0, ncc, kci, pss))
                    items_a.append(ffn2_evict(0, ncc, pss))
                items_b = [ffn1_group(1, fc) for fc in range(8)]
                ia, ib = 0, 0
                while ia < len(items_a) or ib < len(items_b):
                    if ia < len(items_a):
                        items_a[ia]()
                        ia += 1
                    if ib < len(items_b):
                        items_b[ib]()
                        ib += 1

                # [H] LN2 tiles 0-3 + next-layer h^T chain
                h2_tiles = [None] * NT
                if l + 1 < n_layers:
                    nxhT = f8act.tile([P, KD, T], F8, tag="f8",
                                      name=f"xhT{l + 1}")
                    nhbT = bfT.tile([P, KD, T], BF16, tag="bfT",
                                    name=f"hbT{l + 1}")
                for tt in range(4):
                    r = rr_tiles[0][tt]
                    ln_inplace(r[:])
                    h2_tiles[tt] = r
                if l + 1 < n_layers:
                    emit_hT_chain(h2_tiles, nxhT, nhbT, range(4))

                # [I] FFN2 th1
                if l + 1 < n_layers:
                    qk_next = (qk8.tile([P, KD, T], F8, tag="qk8",
                                        name=f"qT{l + 1}"),
                               qk8.tile([P, KD, T], F8, tag="qk8",
                                        name=f"kT{l + 1}"))
                qk0_pending = None
                for ncc in range(2):
                    pss = [psum4.tile([P, 384], F32, tag="ps4",
                                      name=f"pss{l}_1_{ncc}_{j}")
                           for j in range(4)]
                    for kci in range(4):
                        ffn2_chunk(1, ncc, kci, pss)()
                    ffn2_evict(1, ncc, pss)()

                # [J] LN2 tiles 4-7 + next-layer h^T chain
                for tt in range(4):
                    r = rr_tiles[1][tt]
                    ln_inplace(r[:])
                    h2_tiles[4 + tt] = r
                if l + 1 < n_layers:
                    emit_hT_chain(h2_tiles, nxhT, nhbT, range(4, 8))
                    xhT, hbT = nxhT, nhbT
                    cur_w = next_w
                h_tiles = h2_tiles

            # ---- write out ----
            for ti in range(NT):
                nc.sync.dma_start(out=out[ti * P:(ti + 1) * P, :],
                                  in_=h_tiles[ti][:])

    nc.compile()
    return nc


_PROG_CACHE = {}


def _get_program(n_layers=L):
    if n_layers not in _PROG_CACHE:
        _PROG_CACHE[n_layers] = _build_program(n_layers)
    return _PROG_CACHE[n_layers]


def _hilo(w):
    f8 = ml_dtypes.float8_e4m3
    ws = (np.asarray(w, dtype=np.float32) * WS)
    hi = ws.astype(f8)
    lo = (ws - hi.astype(np.float32)).astype(f8)
    return hi, lo


def _prep_inputs(x, segment, tok_emb, seg_emb, Wq, Wk, Wv, Wo, W1, W2,
                 n_layers=L):
    x = np.asarray(x).astype(np.int32)
    segment = np.asarray(segment).astype(np.int32)
    tok_emb = np.asarray(tok_emb, dtype=np.float32)
    seg_emb = np.asarray(seg_emb, dtype=np.float32)
    pe = _positional_table()
    h0_full = tok_emb[x] + seg_emb[segment] + pe[None]  # [B, S, D] f32

    def pack_dd(wf):  # [L, D, D] -> [L, P, 2, KD, D]
        hi, lo = _hilo(wf[:n_layers])
        a = np.stack([hi, lo], axis=1)
        a = a.reshape(n_layers, 2, KD, P, D).transpose(0, 3, 1, 2, 4)
        return np.ascontiguousarray(a)

    wq = pack_dd(Wq)
    wk = pack_dd(Wk)
    wv = pack_dd(Wv)
    wo = pack_dd(Wo)

    hi, lo = _hilo(np.asarray(W1, dtype=np.float32)[:n_layers])
    a = np.stack([hi, lo], axis=1)
    a = a.reshape(n_layers, 2, KD, P, 8, 3, P).transpose(0, 4, 3, 1, 5, 2, 6)
    w1 = np.ascontiguousarray(a)

    hi, lo = _hilo(np.asarray(W2, dtype=np.float32)[:n_layers])
    a = np.stack([hi, lo], axis=1)
    a = a.reshape(n_layers, 2, 4, 6, P, 2, 384)
    a = a.transpose(0, 2, 5, 4, 1, 3, 6)
    w2 = np.ascontiguousarray(a.reshape(n_layers, 8, P, 2, 6, 384))

    bdm = _block_diag_mask()

    shared = {
        "bdm": bdm,
        "wq8": wq, "wk8": wk, "wv8": wv, "wo8": wo, "w18": w1, "w28": w2,
    }
    in_maps = []
    for c in range(NCORES):
        sl = slice(c * BL, (c + 1) * BL)
        m = dict(shared)
        m["x_idx"] = np.ascontiguousarray(x[sl].reshape(T))
        m["h0"] = np.ascontiguousarray(
            h0_full[sl].reshape(T, D).astype(np.float32))
        in_maps.append(m)
    return in_maps


def kernel(x, segment, tok_emb, seg_emb, Wq, bq, Wk, bk, Wv, bv, Wo, bo,
           ln_g, ln_b, W1, b1, W2, b2):
    for name, arr, ref in (("bq", bq, 0.0), ("bk", bk, 0.0), ("bv", bv, 0.0),
                           ("bo", bo, 0.0), ("b1", b1, 0.0), ("b2", b2, 0.0),
                           ("ln_b", ln_b, 0.0), ("ln_g", ln_g, 1.0)):
        a = np.asarray(arr, dtype=np.float32)
        assert np.all(a == ref), f"unsupported nonzero {name}"

    nc = _get_program(L)
    in_maps = _prep_inputs(x, segment, tok_emb, seg_emb, Wq, Wk, Wv, Wo, W1, W2)
    res = run_bass_kernel_spmd(nc, in_maps, list(range(NCORES)))
    parts = [res.results[c]["out"].reshape(BL, S, D) for c in range(NCORES)]
    return np.concatenate(parts, axis=0).astype(np.float32)


# revision 20
# speedup vs baseline: 1.1228x; 1.1228x over previous
"""BERT-base encoder (12 layers) forward for Trainium2, data-parallel over batch.

v3: fp8 DoubleRow GEMMs + XBAR DMA transposes (no PE transposes) + fp8
attention internals + interleaved token-half scheduling.

  - GEMMs in fp8e4 DoubleRow (2 contraction chunks / instr, 0.5 cycles/row):
      QKV:  x_h (w_h + w_l)                    [2-term]
      O/FFN1/FFN2: x_h w_h + x_h w_l + x_l w_h [3-term]
    weights host-prescaled by 32 (lo parts stay normal); 1/32 folded into
    PSUM evictions.
  - all activation transposes (h^T, ctx^T, h1^T) via DMA XBAR transpose
    (bf16), fp8 hi/lo splits from SBUF on Act/DVE/Pool.
  - attention internals fp8 (scores from fp8 q/k, softmax weights and V fp8);
    softmax row-sum fused as a ones-column in V, normalization in eviction.
  - token-half interleave: QKV GEMMs for tokens 0-511 -> attention groups
    0-3 -> QKV for 512-1023 -> groups 4-7, so Act/DVE eviction latency hides
    under PE work.
"""

import numpy as np
import ml_dtypes

import concourse.bass as bass
import concourse.mybir as mybir
import concourse.tile as tile
from concourse import bacc
from concourse.bass_utils import run_bass_kernel_spmd

V, D, L, H, S, B = 30522, 768, 12, 12, 64, 128
DK = D // H            # 64
FF = 4 * D             # 3072
EPS = 1e-5
NCORES = 8
BL = B // NCORES       # 16 sequences per core
T = BL * S             # 1024 tokens per core
P = 128
NT = T // P            # 8 token tiles (= 2-sequence groups)
KD = D // P            # 6 feature tiles
KF = FF // P           # 24 ff tiles
NEG = -1.0e10
WS = 32.0              # host weight pre-scale
ISC = 1.0 / WS
DK1 = DK + 1           # ctx columns + fused rowsum column

F32 = mybir.dt.float32
BF16 = mybir.dt.bfloat16
F8 = mybir.dt.float8e4
I32 = mybir.dt.int32

AF = mybir.ActivationFunctionType
ALU = mybir.AluOpType
DR = mybir.MatmulPerfMode.DoubleRow


def _positional_table():
    pos = np.arange(S, dtype=np.float32)[:, None]
    i = np.arange(0, D, 2, dtype=np.float32)
    arg = pos / (10000.0 ** (2.0 * i / D))
    pe = np.zeros((S, D), dtype=np.float32)
    pe[:, 0::2] = np.sin(arg)
    pe[:, 1::2] = np.cos(arg)
    return pe  # [S, D] f32


def _block_diag_mask():
    m = np.full((P, P), NEG, dtype=np.float32)
    m[:S, :S] = 0.0
    m[S:, S:] = 0.0
    return m


def _build_program(n_layers=L):
    nc = bacc.Bacc("TRN2", target_bir_lowering=False, debug=False,
                   num_devices=NCORES)

    x_idx = nc.dram_tensor("x_idx", [T], I32, kind="ExternalInput").ap()
    h0 = nc.dram_tensor("h0", [T, D], F32, kind="ExternalInput").ap()
    bdm = nc.dram_tensor("bdm", [P, P], F32, kind="ExternalInput").ap()
    wq8 = nc.dram_tensor("wq8", [n_layers, P, 2, KD, D], F8,
                         kind="ExternalInput").ap()
    wk8 = nc.dram_tensor("wk8", [n_layers, P, 2, KD, D], F8,
                         kind="ExternalInput").ap()
    wv8 = nc.dram_tensor("wv8", [n_layers, P, 2, KD, D], F8,
                         kind="ExternalInput").ap()
    wo8 = nc.dram_tensor("wo8", [n_layers, P, 2, KD, D], F8,
                         kind="ExternalInput").ap()
    w18 = nc.dram_tensor("w18", [n_layers, 8, P, 2, 3, KD, P], F8,
                         kind="ExternalInput").ap()
    w28 = nc.dram_tensor("w28", [n_layers, 8, P, 2, 6, 384], F8,
                         kind="ExternalInput").ap()
    out = nc.dram_tensor("out", [T, D], F32, kind="ExternalOutput").ap()

    with tile.TileContext(nc) as tc:
        import contextlib
        ctx = contextlib.ExitStack()
        with ctx:
            const = ctx.enter_context(tc.tile_pool(name="const", bufs=1))
            resid = ctx.enter_context(tc.tile_pool(name="resid", bufs=12))
            f8act = ctx.enter_context(tc.tile_pool(name="f8act", bufs=5))
            qk8 = ctx.enter_context(tc.tile_pool(name="qk8", bufs=2))
            vc = ctx.enter_context(tc.tile_pool(name="vc", bufs=1))
            g8 = ctx.enter_context(tc.tile_pool(name="g8", bufs=2))
            pp = ctx.enter_context(tc.tile_pool(name="pp", bufs=5))
            gB = ctx.enter_context(tc.tile_pool(name="gB", bufs=3))
            hc = ctx.enter_context(tc.tile_pool(name="hc", bufs=2))
            bfT = ctx.enter_context(tc.tile_pool(name="bfT", bufs=1))
            wqk = ctx.enter_context(tc.tile_pool(name="wqk", bufs=2))
            wvo = ctx.enter_context(tc.tile_pool(name="wvo", bufs=2))
            w1p = ctx.enter_context(tc.tile_pool(name="w1p", bufs=2))
            w2p = ctx.enter_context(tc.tile_pool(name="w2p", bufs=3))
            small = ctx.enter_context(tc.tile_pool(name="small", bufs=5))
            psum = ctx.enter_context(
                tc.tile_pool(name="psum", bufs=4, space="PSUM"))
            psum4 = ctx.enter_context(
                tc.tile_pool(name="psum4", bufs=4, space="PSUM"))

            # ---- constants ----
            eps_sb = const.tile([P, 1], F32, tag="eps")
            nc.vector.memset(eps_sb[:], EPS)
            bd_sb = const.tile([P, P], F32, tag="bd")
            nc.sync.dma_start(out=bd_sb[:], in_=bdm[:])

            # ---- embedding (host-precomputed) ----
            h_tiles = []
            for ti in range(NT):
                h = resid.tile([P, D], F32, tag="resid")
                nc.sync.dma_start(out=h[:], in_=h0[ti * P:(ti + 1) * P, :])
                h_tiles.append(h)

            # ---- mask tiles ----
            xg = small.tile([P, NT], I32, tag="xg")
            nc.sync.dma_start(out=xg[:], in_=x_idx.rearrange("(g p) -> p g", p=P))
            am = small.tile([P, NT], F32, tag="am")
            nc.vector.tensor_scalar(out=am[:], in0=xg[:], scalar1=0, scalar2=None,
                                    op0=ALU.is_gt)
            nc.vector.tensor_scalar(out=am[:], in0=am[:], scalar1=1.0,
                                    scalar2=-NEG, op0=ALU.subtract, op1=ALU.mult)
            maskt = const.tile([P, NT, P], BF16, tag="maskt")
            for g in range(NT):
                nc.vector.tensor_scalar(out=maskt[:, g, :], in0=bd_sb[:],
                                        scalar1=am[:, g:g + 1], scalar2=None,
                                        op0=ALU.add)

            def ln_inplace(r):
                st = small.tile([P, 3, 6], F32, tag="st")
                for sg in range(3):
                    nc.vector.bn_stats(out=st[:, sg, :],
                                       in_=r[:, sg * 256:(sg + 1) * 256])
                mv = small.tile([P, 2], F32, tag="mv")
                nc.vector.bn_aggr(out=mv[:], in_=st[:])
                rstd = small.tile([P, 1], F32, tag="rstd")
                nc.scalar.activation(out=rstd[:], in_=mv[:, 1:2],
                                     func=AF.Sqrt, bias=eps_sb[:])
                nc.vector.reciprocal(out=rstd[:], in_=rstd[:])
                nc.vector.tensor_scalar(out=r[:], in0=r[:],
                                        scalar1=mv[:, 0:1], scalar2=rstd[:],
                                        op0=ALU.subtract, op1=ALU.mult)

            # ---- transformer layers ----
            # Software-pipelined emission: attention is split into
            # scores-halves and PV-halves with QKV matmul groups as filler,
            # FFN2(th0) interleaves with FFN1(th1), LN2 is deferred to keep
            # the Act table sequence at Exp|Sqrt|Gelu|Sqrt (4 loads/layer),
            # and next-layer weights + h^T chains are emitted inside the FFN
            # phase so every phase boundary has PE work queued.

            def load_wqkvo(l):
                tiles = {}
                for nm, ap in (("q", wq8), ("k", wk8)):
                    t = wqk.tile([P, 2, KD, D], F8, tag="wqk")
                    nc.sync.dma_start(out=t[:], in_=ap[l])
                    tiles[nm] = t
                for nm, ap in (("v", wv8), ("o", wo8)):
                    t = wvo.tile([P, 2, KD, D], F8, tag="wvo")
                    nc.sync.dma_start(out=t[:], in_=ap[l])
                    tiles[nm] = t
                return tiles

            def emit_hT_chain(h_tiles, xhT, hbT, tis):
                for ti in tis:
                    hb = hc.tile([P, D], BF16, tag="hc")
                    nc.gpsimd.tensor_copy(out=hb[:], in_=h_tiles[ti][:])
                    sl = slice(ti * P, (ti + 1) * P)
                    nc.sync.dma_start(out=hbT[:, :, sl], in_=hb[:],
                                      transpose=True)
                    nc.gpsimd.tensor_copy(out=xhT[:, :, sl],
                                          in_=hbT[:, :, sl])

            cur_w = load_wqkvo(0)
            xhT = f8act.tile([P, KD, T], F8, tag="f8", name="xhT0")
            hbT = bfT.tile([P, KD, T], BF16, tag="bfT", name="hbT0")
            emit_hT_chain(h_tiles, xhT, hbT, range(NT))
            qk_next = (qk8.tile([P, KD, T], F8, tag="qk8", name="qT0"),
                       qk8.tile([P, KD, T], F8, tag="qk8", name="kT0"))
            qk0_pending = None

            for l in range(n_layers):
                qTa, kTa = qk_next
                Vta = vc.tile([P, NT, H * DK1], F8, tag="vta", name=f"Vt{l}")
                ones_v = Vta[:].rearrange("p g (h c) -> p g h c", c=DK1)
                nc.vector.memset(ones_v[:, :, :, DK:DK1], 1.0)
                cta = vc.tile([P, NT, D], BF16, tag="cta", name=f"ctx{l}")
                ch = f8act.tile([P, KD, T], F8, tag="f8", name=f"ch{l}")
                cl = f8act.tile([P, KD, T], F8, tag="f8", name=f"cl{l}")
                cbT = bfT.tile([P, KD, T], BF16, tag="bfT", name=f"cbT{l}")

                def qk_group(w_sb, dstT, j, tc2, xsrc, on_dve=False):
                    def go():
                        tok = slice(tc2 * 512, (tc2 + 1) * 512)
                        ps = psum.tile([P, 512], F32, tag="ps")
                        for v in range(2):
                            for kp in range(3):
                                nc.tensor.matmul(
                                    out=ps[:],
                                    lhsT=w_sb[:, v, 2 * kp:2 * kp + 2,
                                              j * P:(j + 1) * P],
                                    rhs=xsrc[:, 2 * kp:2 * kp + 2, tok],
                                    start=(v == 0 and kp == 0),
                                    stop=(v == 1 and kp == 2),
                                    perf_mode=DR)
                        if on_dve:
                            nc.vector.tensor_scalar(
                                out=dstT[:, j, tok], in0=ps[:],
                                scalar1=ISC, scalar2=None, op0=ALU.mult)
                        else:
                            nc.scalar.activation(out=dstT[:, j, tok],
                                                 in_=ps[:],
                                                 func=AF.Copy, scale=ISC)
                    return go

                def v_group(ti, ncc):
                    def go():
                        ps = psum.tile([P, 384], F32, tag="ps")
                        for v in range(2):
                            for kp in range(3):
                                nc.tensor.matmul(
                                    out=ps[:],
                                    lhsT=xhT[:, 2 * kp:2 * kp + 2,
                                             ti * P:(ti + 1) * P],
                                    rhs=cur_w["v"][:, v, 2 * kp:2 * kp + 2,
                                                   ncc * 384:(ncc + 1) * 384],
                                    start=(v == 0 and kp == 0),
                                    stop=(v == 1 and kp == 2),
                                    perf_mode=DR)
                        nc.vector.tensor_scalar(
                            out=ones_v[:, ti, 6 * ncc:6 * ncc + 6, 0:DK],
                            in0=ps[:].rearrange("p (h c) -> p h c", c=DK),
                            scalar1=ISC, scalar2=None, op0=ALU.mult)
                    return go

                BLKS = ((0, 0, 4), (0, 4, 2), (1, 0, 4), (1, 4, 2))

                def make_attn(g, bi):
                    two, pr0, npr = BLKS[bi]
                    po = two * DK
                    w_ = npr * P
                    st = {}

                    def sc():
                        sps = psum.tile([P, w_], F32, tag="ps",
                                        name=f"sps{l}_{g}_{bi}")
                        for i in range(npr):
                            jt = pr0 + i
                            nc.tensor.matmul(
                                out=sps[:, i * P:(i + 1) * P],
                                lhsT=kTa[po:po + DK, jt, g * P:(g + 1) * P],
                                rhs=qTa[po:po + DK, jt, g * P:(g + 1) * P],
                                start=True, stop=True)
                        sps4 = sps[:].rearrange("p (i c) -> p i c", i=npr)
                        nc.vector.tensor_tensor(
                            out=sps4, in0=sps4,
                            in1=maskt[:, g, None, :].to_broadcast([P, npr, P]),
                            op=ALU.add)
                        pT = pp.tile([P, 512], F8, tag="pt4")
                        nc.scalar.activation(out=pT[:, :w_], in_=sps[:],
                                             func=AF.Exp, scale=0.125)
                        st["pT"] = pT

                    def pv():
                        pT = st["pT"]
                        cps = psum4.tile([P, npr * DK1], F32, tag="ps4",
                                         name=f"cps{l}_{g}_{bi}")
                        for i in range(npr):
                            hh = 2 * (pr0 + i) + two
                            nc.tensor.matmul(
                                out=cps[:, i * DK1:(i + 1) * DK1],
                                lhsT=pT[:, i * P:(i + 1) * P],
                                rhs=Vta[:, g, hh * DK1:(hh + 1) * DK1],
                                start=True, stop=True)
                        cpsv = cps[:].rearrange("p (i c) -> p i c", c=DK1)
                        rsi = small.tile([P, 4], F32, tag="rsi")
                        nc.vector.reciprocal(out=rsi[:, :npr],
                                             in_=cpsv[:, :, DK])
                        cta_v = cta[:, g, :].rearrange(
                            "p (pr two c) -> p two pr c", two=2, c=DK)
                        nc.vector.tensor_tensor(
                            out=cta_v[:, two, pr0:pr0 + npr, :],
                            in0=cpsv[:, :, 0:DK],
                            in1=rsi[:, :npr, None].to_broadcast([P, npr, DK]),
                            op=ALU.mult)
                        if bi == 3:
                            sl = slice(g * P, (g + 1) * P)
                            nc.sync.dma_start(out=cbT[:, :, sl],
                                              in_=cta[:, g, :], transpose=True)
                            nc.scalar.copy(out=ch[:, :, sl],
                                           in_=cbT[:, :, sl])
                            nc.vector.tensor_tensor(out=cl[:, :, sl],
                                                    in0=cbT[:, :, sl],
                                                    in1=ch[:, :, sl],
                                                    op=ALU.subtract)
                    return sc, pv

                def attn_pipe(gs, fill):
                    blocks = [make_attn(g, bi) for g in gs for bi in range(4)]
                    n = len(blocks)
                    fi = iter(fill)
                    blocks[0][0]()
                    blocks[1][0]()
                    blocks[2][0]()
                    for i in range(n):
                        if i % 2 == 0:
                            f = next(fi, None)
                            if f is not None:
                                f()
                        if i + 3 < n:
                            blocks[i + 3][0]()
                        blocks[i][1]()
                    for f in fi:
                        f()

                # [A/B] QKV for token half 0 (layer 0 emits here; later
                # layers emit these as FFN2-th1 fill of the previous layer)
                if qk0_pending is None:
                    qk0_pending = [qk_group(cur_w["q"], qTa, j, 0, xhT,
                                            False) for j in range(KD)]
                    qk0_pending += [qk_group(cur_w["k"], kTa, j, 0, xhT,
                                             False) for j in range(KD)]
                for go in qk0_pending:
                    go()
                for ti in range(4):
                    for ncc in range(2):
                        v_group(ti, ncc)()
                # [C] attention 0-3 with QK half-1 as filler
                fill = [qk_group(cur_w["q"], qTa, j, 1, xhT, False)
                        for j in range(KD)]
                fill += [qk_group(cur_w["k"], kTa, j, 1, xhT, False)
                         for j in range(KD)]
                attn_pipe(range(4), fill)
                # [D] attention 4-7 with V half-1 as filler
                fill = [v_group(ti, ncc) for ti in range(4, 8)
                        for ncc in range(2)]
                attn_pipe(range(4, 8), fill)

                # [E] O-projection + LN1 + h1^T chain per tile
                xh1 = f8act.tile([P, KD, T], F8, tag="f8", name=f"xh1{l}")
                xl1 = f8act.tile([P, KD, T], F8, tag="f8", name=f"xl1{l}")
                h1bT = bfT.tile([P, KD, T], BF16, tag="bfT", name=f"h1bT{l}")

                w1_seq = [fc for fc in range(8)] * 2
                w1_slots = [None] * len(w1_seq)

                def ensure_w1(pos):
                    if pos < len(w1_seq) and w1_slots[pos] is None:
                        t = w1p.tile([P, 2, 3, KD, P], F8, tag="w1")
                        nc.sync.dma_start(out=t[:], in_=w18[l, w1_seq[pos]])
                        w1_slots[pos] = t

                w2_seq = [kc * 2 + ncc for th in range(2)
                          for ncc in range(2) for kc in range(4)]
                w2_slots = [None] * len(w2_seq)

                def ensure_w2(pos):
                    if pos < len(w2_seq) and w2_slots[pos] is None:
                        t = w2p.tile([P, 2, 6, 384], F8, tag="w2")
                        nc.sync.dma_start(out=t[:], in_=w28[l, w2_seq[pos]])
                        w2_slots[pos] = t

                ensure_w1(0)
                ensure_w1(1)
                h1_tiles = []
                for ti in range(NT):
                    r = resid.tile([P, D], F32, tag="resid")
                    for ncc in range(2):
                        ps = psum.tile([P, 384], F32, tag="ps")
                        for a, (xt, v) in enumerate(
                                ((ch, 0), (cl, 0), (ch, 1))):
                            for kp in range(3):
                                nc.tensor.matmul(
                                    out=ps[:],
                                    lhsT=xt[:, 2 * kp:2 * kp + 2,
                                            ti * P:(ti + 1) * P],
                                    rhs=cur_w["o"][:, v, 2 * kp:2 * kp + 2,
                                                   ncc * 384:(ncc + 1) * 384],
                                    start=(a == 0 and kp == 0),
                                    stop=(a == 2 and kp == 2),
                                    perf_mode=DR)
                        nc.vector.scalar_tensor_tensor(
                            out=r[:, ncc * 384:(ncc + 1) * 384],
                            in0=ps[:], scalar=ISC,
                            in1=h_tiles[ti][:, ncc * 384:(ncc + 1) * 384],
                            op0=ALU.mult, op1=ALU.add)
                    ln_inplace(r[:])
                    h1_tiles.append(r)
                    hb = hc.tile([P, D], BF16, tag="hc")
                    nc.gpsimd.tensor_copy(out=hb[:], in_=r[:])
                    sl = slice(ti * P, (ti + 1) * P)
                    nc.sync.dma_start(out=h1bT[:, :, sl], in_=hb[:],
                                      transpose=True)
                    nc.scalar.copy(out=xh1[:, :, sl], in_=h1bT[:, :, sl])
                    nc.vector.tensor_tensor(out=xl1[:, :, sl],
                                            in0=h1bT[:, :, sl],
                                            in1=xh1[:, :, sl],
                                            op=ALU.subtract)

                # FFN pieces
                def ffn1_group(th, fc):
                    def go():
                        tok = slice(th * 512, (th + 1) * 512)
                        w1pos = th * 8 + fc
                        ensure_w1(w1pos + 1)
                        w1c = w1_slots[w1pos]
                        gh8, gl8 = g_tiles[th]
                        for ff in range(3):
                            ft = fc * 3 + ff
                            ps = psum.tile([P, 512], F32, tag="ps")
                            for a, (xt, v) in enumerate(
                                    ((xh1, 0), (xh1, 1), (xl1, 0))):
                                for kp in range(3):
                                    nc.tensor.matmul(
                                        out=ps[:],
                                        lhsT=w1c[:, v, ff,
                                                 2 * kp:2 * kp + 2, :],
                                        rhs=xt[:, 2 * kp:2 * kp + 2, tok],
                                        start=(a == 0 and kp == 0),
                                        stop=(a == 2 and kp == 2),
                                        perf_mode=DR)
                            gBt = gB.tile([P, 512], BF16, tag="gB")
                            nc.scalar.activation(out=gBt[:], in_=ps[:],
                                                 func=AF.Gelu, scale=ISC)
                            nc.scalar.activation(out=gh8[:, ft, :], in_=ps[:],
                                                 func=AF.Gelu, scale=ISC)
                            nc.gpsimd.tensor_sub(out=gl8[:, ft, :],
                                                 in0=gBt[:],
                                                 in1=gh8[:, ft, :])
                    return go

                def ffn2_chunk(th, ncc, kci, pss):
                    def go():
                        w2pos = (th * 2 + ncc) * 4 + kci
                        ensure_w2(w2pos + 1)
                        ensure_w2(w2pos + 2)
                        w2c = w2_slots[w2pos]
                        i8 = w2_seq[w2pos]
                        gh8, gl8 = g_tiles[th]
                        for kkp in range(3):
                            first = (kci == 0 and kkp == 0)
                            last = (kci == 3 and kkp == 2)
                            kt = (i8 // 2) * 6 + 2 * kkp
                            for tt in range(4):
                                for a, (gt, v) in enumerate(
                                        ((gh8, 0), (gl8, 0), (gh8, 1))):
                                    nc.tensor.matmul(
                                        out=pss[tt][:],
                                        lhsT=gt[:, kt:kt + 2,
                                                tt * P:(tt + 1) * P],
                                        rhs=w2c[:, v, 2 * kkp:2 * kkp + 2, :],
                                        start=(first and a == 0),
                                        stop=(last and a == 2),
                                        perf_mode=DR)
                    return go

                def ffn2_evict(th, ncc, pss):
                    def go():
                        for tt in range(4):
                            ti = th * 4 + tt
                            nc.vector.scalar_tensor_tensor(
                                out=rr_tiles[th][tt][:,
                                                     ncc * 384:(ncc + 1) * 384],
                                in0=pss[tt][:], scalar=ISC,
                                in1=h1_tiles[ti][:,
                                                 ncc * 384:(ncc + 1) * 384],
                                op0=ALU.mult, op1=ALU.add)
                    return go

                g_tiles = {}
                rr_tiles = {}
                for th in range(2):
                    g_tiles[th] = (
                        g8.tile([P, KF, 512], F8, tag="g8", name=f"gh{l}_{th}"),
                        g8.tile([P, KF, 512], F8, tag="g8", name=f"gl{l}_{th}"))
                    rr_tiles[th] = [resid.tile([P, D], F32, tag="resid",
                                               name=f"rr{l}_{th}_{tt}")
                                    for tt in range(4)]
                ensure_w2(0)
                ensure_w2(1)

                # [F] FFN1 th0
                for fc in range(8):
                    ffn1_group(0, fc)()

                # [G] FFN2 th0 interleaved with FFN1 th1; prefetch next-layer
                # weights at the start
                if l + 1 < n_layers:
                    next_w = load_wqkvo(l + 1)
                items_a = []
                pss_hold = {}
                for ncc in range(2):
                    pss = [psum4.tile([P, 384], F32, tag="ps4",
                                      name=f"pss{l}_0_{ncc}_{j}")
                           for j in range(4)]
                    pss_hold[ncc] = pss
                    for kci in range(4):
                        items_a.append(ffn2_chunk(0, ncc, kci, pss))
                    items_a.append(ffn2_evict(0, ncc, pss))
                items_b = [ffn1_group(1, fc) for fc in range(8)]
                ia, ib = 0, 0
                while ia < len(items_a) or ib < len(items_b):
                    if ia < len(items_a):
                        items_a[ia]()
                        ia += 1
                    if ib < len(items_b):
                        items_b[ib]()
                        ib += 1

                # [H] LN2 tiles 0-3 + next-layer h^T chain
                h2_tiles = [None] * NT
                if l + 1 < n_layers:
                    nxhT = f8act.tile([P, KD, T], F8, tag="f8",
                                      name=f"xhT{l + 1}")
                    nhbT = bfT.tile([P, KD, T], BF16, tag="bfT",
                                    name=f"hbT{l + 1}")
                for tt in range(4):
                    r = rr_tiles[0][tt]
                    ln_inplace(r[:])
                    h2_tiles[tt] = r
                if l + 1 < n_layers:
                    emit_hT_chain(h2_tiles, nxhT, nhbT, range(4))

                # [I] FFN2 th1
                if l + 1 < n_layers:
                    qk_next = (qk8.tile([P, KD, T], F8, tag="qk8",
                                        name=f"qT{l + 1}"),
                               qk8.tile([P, KD, T], F8, tag="qk8",
                                        name=f"kT{l + 1}"))
                qk0_pending = None
                for ncc in range(2):
                    pss = [psum4.tile([P, 384], F32, tag="ps4",
                                      name=f"pss{l}_1_{ncc}_{j}")
                           for j in range(4)]
                    for kci in range(4):
                        ffn2_chunk(1, ncc, kci, pss)()
                    ffn2_evict(1, ncc, pss)()

                # [J] LN2 tiles 4-7 + next-layer h^T chain
                for tt in range(4):
                    r = rr_tiles[1][tt]
                    ln_inplace(r[:])
                    h2_tiles[4 + tt] = r
                if l + 1 < n_layers:
                    emit_hT_chain(h2_tiles, nxhT, nhbT, range(4, 8))
                    xhT, hbT = nxhT, nhbT
                    cur_w = next_w
                h_tiles = h2_tiles

            # ---- write out ----
            for ti in range(NT):
                nc.sync.dma_start(out=out[ti * P:(ti + 1) * P, :],
                                  in_=h_tiles[ti][:])

    nc.compile()
    return nc


_PROG_CACHE = {}


def _get_program(n_layers=L):
    if n_layers not in _PROG_CACHE:
        _PROG_CACHE[n_layers] = _build_program(n_layers)
    return _PROG_CACHE[n_layers]


def _hilo(w):
    f8 = ml_dtypes.float8_e4m3
    ws = (np.asarray(w, dtype=np.float32) * WS)
    hi = ws.astype(f8)
    lo = (ws - hi.astype(np.float32)).astype(f8)
    return hi, lo


def _prep_inputs(x, segment, tok_emb, seg_emb, Wq, Wk, Wv, Wo, W1, W2,
                 n_layers=L):
    x = np.asarray(x).astype(np.int32)
    segment = np.asarray(segment).astype(np.int32)
    tok_emb = np.asarray(tok_emb, dtype=np.float32)
    seg_emb = np.asarray(seg_emb, dtype=np.float32)
    pe = _positional_table()
    h0_full = tok_emb[x] + seg_emb[segment] + pe[None]  # [B, S, D] f32

    def pack_dd(wf):  # [L, D, D] -> [L, P, 2, KD, D]
        hi, lo = _hilo(wf[:n_layers])
        a = np.stack([hi, lo], axis=1)
        a = a.reshape(n_layers, 2, KD, P, D).transpose(0, 3, 1, 2, 4)
        return np.ascontiguousarray(a)

    wq = pack_dd(Wq)
    wk = pack_dd(Wk)
    wv = pack_dd(Wv)
    wo = pack_dd(Wo)

    hi, lo = _hilo(np.asarray(W1, dtype=np.float32)[:n_layers])
    a = np.stack([hi, lo], axis=1)
    a = a.reshape(n_layers, 2, KD, P, 8, 3, P).transpose(0, 4, 3, 1, 5, 2, 6)
    w1 = np.ascontiguousarray(a)

    hi, lo = _hilo(np.asarray(W2, dtype=np.float32)[:n_layers])
    a = np.stack([hi, lo], axis=1)
    a = a.reshape(n_layers, 2, 4, 6, P, 2, 384)
    a = a.transpose(0, 2, 5, 4, 1, 3, 6)
    w2 = np.ascontiguousarray(a.reshape(n_layers, 8, P, 2, 6, 384))

    bdm = _block_diag_mask()

    shared = {
        "bdm": bdm,
        "wq8": wq, "wk8": wk, "wv8": wv, "wo8": wo, "w18": w1, "w28": w2,
    }
    in_maps = []
    for c in range(NCORES):
        sl = slice(c * BL, (c + 1) * BL)
        m = dict(shared)
        m["x_idx"] = np.ascontiguousarray(x[sl].reshape(T))
        m["h0"] = np.ascontiguousarray(
            h0_full[sl].reshape(T, D).astype(np.float32))
        in_maps.append(m)
    return in_maps


def kernel(x, segment, tok_emb, seg_emb, Wq, bq, Wk, bk, Wv, bv, Wo, bo,
           ln_g, ln_b, W1, b1, W2, b2):
    for name, arr, ref in (("bq", bq, 0.0), ("bk", bk, 0.0), ("bv", bv, 0.0),
                           ("bo", bo, 0.0), ("b1", b1, 0.0), ("b2", b2, 0.0),
                           ("ln_b", ln_b, 0.0), ("ln_g", ln_g, 1.0)):
        a = np.asarray(arr, dtype=np.float32)
        assert np.all(a == ref), f"unsupported nonzero {name}"

    nc = _get_program(L)
    in_maps = _prep_inputs(x, segment, tok_emb, seg_emb, Wq, Wk, Wv, Wo, W1, W2)
    res = run_bass_kernel_spmd(nc, in_maps, list(range(NCORES)))
    parts = [res.results[c]["out"].reshape(BL, S, D) for c in range(NCORES)]
    return np.concatenate(parts, axis=0).astype(np.float32)
